# revision 1
# baseline (speedup 1.0000x reference)
"""GAT (2-layer, PyG GATConv semantics) on 8 Trainium2 NeuronCores.

Strategy (dst-sharded edge parallelism):
  - Append self-loops, sort edges by dst. Core k owns dst nodes
    [k*2560, (k+1)*2560) (N padded 20000 -> 20480), as 20 blocks of 128.
  - Every core redundantly computes h = x@W1 (+ per-node attention logits
    alpha_src/alpha_dst, fused as extra matmul columns) and writes a packed
    row table to its HBM. Edge processing gathers h[src_e] rows with
    dma_gather, builds per-tile one-hot matrices from dst_local indices, and
    uses PE matmuls to (a) broadcast alpha_dst[dst] to edges and (b)
    scatter-add softmax-weighted messages + softmax denominators into PSUM.
  - Softmax is computed without the max-subtraction (logits are O(1) here;
    mathematically identical).
  - One AllGather exchanges packed layer-2 features (g = ELU(y1)@W2 plus
    logits) between the layers. Final log_softmax per dst row on-chip.
"""

import math
import os

import numpy as np

# ---- problem constants (hardcoded per contract) ----
N = 20000
F = 128
HEADS = 8
CH = 32
HC = HEADS * CH  # 256
CLS = 40
E0 = 640000
NEG = 0.2
CORES = 8
BLK = 128
BPC = 20  # blocks per core
NPC = BLK * BPC  # 2560 nodes per core
NPAD = NPC * CORES  # 20480
HP_W = 320  # packed h row: [h(256) | a_src(8) | a_dst(8) | pad] -> 1280B
GP_W = 64  # packed g row: [g(40) | as2(1) | ad2(1) | pad] -> 256B
AD_W = 64  # alpha_dst table row (8 used)

_cache = {}


def _wrap_idx(idx):
    """dma_gather index layout: [128, len//16] int16, idx i at [i%16, i//16],
    replicated across the 8 gpsimd-core partition groups."""
    assert len(idx) % 16 == 0
    w = idx.astype(np.int16).reshape(-1, 16).T  # [16, len//16]
    return np.tile(w, (8, 1))  # [128, len//16]


def _prep_edges(edge_index):
    src = np.asarray(edge_index[0], dtype=np.int64)
    dst = np.asarray(edge_index[1], dtype=np.int64)
    loops = np.arange(N, dtype=np.int64)
    src = np.concatenate([src, loops])
    dst = np.concatenate([dst, loops])
    order = np.argsort(dst, kind="stable")
    ssrc = src[order]
    sdst = dst[order]

    nblocks = NPAD // BLK  # 160
    counts = np.bincount(sdst // BLK, minlength=nblocks)
    starts = np.concatenate([[0], np.cumsum(counts)])
    # uniform 12-tile chunks (single num_idxs constant -> one gpsimd register)
    CN = 7
    tmax = CN * int(math.ceil(counts.max() / 128 / CN))
    chunks = [CN] * (tmax // CN)

    per_core = []
    for k in range(CORES):
        gsrc_cols = []
        dstl_cols = np.empty((BPC * tmax, 128), dtype=np.float32)
        for b in range(BPC):
            g = k * BPC + b
            e0, e1 = starts[g], starts[g + 1]
            npadded = tmax * 128
            s = np.zeros(npadded, dtype=np.int64)
            dl = np.full(npadded, 128.0, dtype=np.float32)  # 128 = dead sentinel
            s[: e1 - e0] = ssrc[e0:e1]
            dl[: e1 - e0] = (sdst[e0:e1] - g * BLK).astype(np.float32)
            dstl_cols[b * tmax : (b + 1) * tmax] = dl.reshape(tmax, 128)
            # per-chunk wrapped gather indices
            t0 = 0
            for cn in chunks:
                gsrc_cols.append(_wrap_idx(s[t0 * 128 : (t0 + cn) * 128]))
                t0 += cn
        gsrc = np.concatenate(gsrc_cols, axis=1)  # [128, BPC*tmax*8]
        gdstl = np.ascontiguousarray(dstl_cols.T)  # [128, BPC*tmax]
        ids = np.arange(k * NPC, (k + 1) * NPC, dtype=np.int64)
        gs = min(512, NPC)
        gblk = np.concatenate(
            [_wrap_idx(ids[p * gs : (p + 1) * gs]) for p in range(NPC // gs)], axis=1
        )
        per_core.append({"gsrc": gsrc, "gdstl": gdstl, "gblk": gblk})
    return tmax, chunks, per_core


def _build_nc(tmax, chunks):
    import concourse.bacc as bacc
    import concourse.bass as bass
    import concourse.mybir as mybir
    import concourse.tile as tile

    fp32 = mybir.dt.float32
    i16 = mybir.dt.int16
    ALU = mybir.AluOpType
    ACT = mybir.ActivationFunctionType

    nc = bacc.Bacc("TRN2", target_bir_lowering=False, num_swdge_queues=4)

    # ---- I/O ----
    x_t = nc.dram_tensor("x", [NPAD, F], fp32, kind="ExternalInput")
    w1_t = nc.dram_tensor("W1", [F, HC], fp32, kind="ExternalInput")
    ablk_t = nc.dram_tensor("Ablk", [HC, 16], fp32, kind="ExternalInput")
    b1_t = nc.dram_tensor("b1rep", [128, HC], fp32, kind="ExternalInput")
    w2_t = nc.dram_tensor("W2", [HC, CLS], fp32, kind="ExternalInput")
    w2T_t = nc.dram_tensor("W2T", [CLS, HC], fp32, kind="ExternalInput")
    a2_t = nc.dram_tensor("a2sd", [CLS, 2], fp32, kind="ExternalInput")
    b2_t = nc.dram_tensor("b2rep", [128, CLS], fp32, kind="ExternalInput")
    ident_t = nc.dram_tensor("ident", [128, 128], fp32, kind="ExternalInput")
    iota_t = nc.dram_tensor("iotaF", [128, 128], fp32, kind="ExternalInput")
    gsrc_t = nc.dram_tensor("gsrc", [128, BPC * tmax * 8], i16, kind="ExternalInput")
    gdstl_t = nc.dram_tensor("gdstl", [128, BPC * tmax], fp32, kind="ExternalInput")
    gblk_t = nc.dram_tensor("gblk", [128, NPC // 16], i16, kind="ExternalInput")
    out_t = nc.dram_tensor("out", [NPC, CLS], fp32, kind="ExternalOutput")

    hpack_t = nc.dram_tensor("hpack", [NPAD, HP_W], fp32)
    adt_t = nc.dram_tensor("adt", [NPAD, AD_W], fp32)
    gpk_in_t = nc.dram_tensor("gpk_in", [NPC, GP_W], fp32)
    gpk_t = nc.dram_tensor("gpk", [NPAD, GP_W], fp32, addr_space="Shared")

    ntile = NPAD // 128  # 160

    with tile.TileContext(nc) as tc:
        with (
            tc.tile_pool(name="const", bufs=1) as cp,
            tc.tile_pool(name="sb", bufs=2) as sb,
            tc.tile_pool(name="oh", bufs=2 * max(chunks)) as ohp,
            tc.tile_pool(name="res", bufs=1) as res,
        ):
            # ---- load constants ----
            ident = cp.tile([128, 128], fp32)
            nc.sync.dma_start(ident[:], ident_t[:])
            iota = cp.tile([128, 128], fp32)
            nc.sync.dma_start(iota[:], iota_t[:])
            w1 = cp.tile([128, HC], fp32)
            nc.sync.dma_start(w1[:], w1_t[:])
            ablk = cp.tile([128, 2, 16], fp32)
            nc.sync.dma_start(ablk[:], ablk_t[:].rearrange("(h p) a -> p h a", p=128))
            b1r = cp.tile([128, HC], fp32)
            nc.sync.dma_start(b1r[:], b1_t[:])
            w2h = cp.tile([128, 2, CLS], fp32)
            nc.sync.dma_start(w2h[:], w2_t[:].rearrange("(h p) a -> p h a", p=128))
            w2T = cp.tile([CLS, HC], fp32)
            nc.sync.dma_start(w2T[:], w2T_t[:])
            a2 = cp.tile([CLS, 2], fp32)
            nc.sync.dma_start(a2[:], a2_t[:])
            b2r = cp.tile([128, CLS], fp32)
            nc.sync.dma_start(b2r[:], b2_t[:])
            gsrc = cp.tile([128, BPC * tmax * 8], i16)
            nc.sync.dma_start(gsrc[:], gsrc_t[:])
            gblk = cp.tile([128, NPC // 16], i16)
            nc.sync.dma_start(gblk[:], gblk_t[:])

            # ---- W1T (2 halves) then U = W1 @ Ablk  -> rhs_ext [128,272] ----
            ps = tc.alloc_tile_pool(name="ps_setup", bufs=2, space="PSUM")
            w1T = cp.tile([128, 2, 128], fp32)
            rhs_ext = cp.tile([128, HC + 16], fp32)
            nc.scalar.activation(rhs_ext[:, 0:HC], w1[:], ACT.Copy)
            for h in range(2):
                tp = ps.tile([128, 128], fp32, tag="tps")
                nc.tensor.transpose(tp[:], w1[:, h * 128 : (h + 1) * 128], ident[:])
                nc.scalar.activation(w1T[:, h, :], tp[:], ACT.Copy)
            ups = ps.tile([128, 16], fp32, tag="ups")
            for h in range(2):
                nc.tensor.matmul(
                    ups[:], lhsT=w1T[:, h, :], rhs=ablk[:, h, :],
                    start=(h == 0), stop=(h == 1),
                )
            nc.scalar.activation(rhs_ext[:, HC : HC + 16], ups[:], ACT.Copy)

            # ---- rhs2 halves: [W2_half | va_half | vb_half] [128, 2, 42] ----
            rhs2 = cp.tile([128, 2, CLS + 2], fp32)
            for h in range(2):
                vab = ps.tile([128, 2], fp32, tag="vab")
                nc.tensor.matmul(
                    vab[:], lhsT=w2T[:, h * 128 : (h + 1) * 128], rhs=a2[:],
                    start=True, stop=True,
                )
                nc.scalar.activation(rhs2[:, h, 0:CLS], w2h[:, h, :], ACT.Copy)
                nc.scalar.activation(rhs2[:, h, CLS : CLS + 2], vab[:], ACT.Copy)

            ps.release()
            # ---- prologue: h | a_s | a_d for all nodes -> hpack, adt ----
            ps = tc.alloc_tile_pool(name="ps_pro", bufs=2, space="PSUM")
            for t in range(ntile):
                xt = sb.tile([128, F], fp32, tag="xt")
                nc.sync.dma_start(xt[:], x_t[t * 128 : (t + 1) * 128, :])
                xT_ps = ps.tile([128, 128], fp32, tag="xT")
                nc.tensor.transpose(xT_ps[:], xt[:], ident[:])
                xT = sb.tile([128, 128], fp32, tag="xTs")
                nc.scalar.activation(xT[:], xT_ps[:], ACT.Copy)
                hps = ps.tile([128, HC + 16], fp32, tag="hps")
                nc.tensor.matmul(hps[:], lhsT=xT[:], rhs=rhs_ext[:], start=True, stop=True)
                hp = sb.tile([128, HP_W], fp32, tag="hp")
                nc.scalar.activation(hp[:, 0 : HC + 16], hps[:], ACT.Copy)
                nc.vector.memset(hp[:, HC + 16 : HP_W], 0.0)
                nc.sync.dma_start(hpack_t[t * 128 : (t + 1) * 128, :], hp[:])
                adp = sb.tile([128, AD_W], fp32, tag="adp")
                nc.scalar.activation(adp[:, 0:8], hps[:, HC + 8 : HC + 16], ACT.Copy)
                nc.vector.memset(adp[:, 8:AD_W], 0.0)
                nc.sync.dma_start(adt_t[t * 128 : (t + 1) * 128, :], adp[:])

            # ---- own-block alpha_dst via data-driven block gather ----
            # pre-allocated gpsimd registers (the free pool is tiny under Tile)
            cnk_reg = nc.gpsimd.to_reg(chunks[0] * 128)
            adblk = res.tile([128, BPC, AD_W], fp32)
            GS = min(512, NPC)
            for p in range(NPC // GS):
                nc.gpsimd.dma_gather(
                    adblk[:, p * (GS // 128) : (p + 1) * (GS // 128), :], adt_t[:],
                    gblk[:, p * (GS // 16) : (p + 1) * (GS // 16)], GS, GS, AD_W,
                    queue_num=0,
                )

            h2res = res.tile([128, BPC, HC], fp32)
            ps.release()
            ps = tc.alloc_tile_pool(name="ps_l1", bufs=2, space="PSUM")

            # ================= layer 1 edge phase =================
            for b in range(BPC):
                agg = ps.tile([128, HC + 8], fp32, tag="agg")
                dstl = sb.tile([128, tmax], fp32, tag="dstl")
                nc.sync.dma_start(dstl[:], gdstl_t[:, b * tmax : (b + 1) * tmax])
                t0 = 0
                for cn in chunks:
                    col0 = (b * tmax + t0) * 8
                    hg = sb.tile([128, cn, HP_W], fp32, tag="hg")
                    nc.gpsimd.dma_gather(
                        hg[:], hpack_t[:], gsrc[:, col0 : col0 + cn * 8],
                        cn * 128, cnk_reg, HP_W, queue_num=0,
                    )
                    ohs = []
                    adeps = ps.tile([128, cn * 8], fp32, tag="adeps")
                    for j in range(cn):
                        oh = ohp.tile([128, 128], fp32, tag="oh")
                        nc.vector.tensor_tensor(
                            out=oh[:],
                            in0=dstl[:, t0 + j : t0 + j + 1].to_broadcast([128, 128]),
                            in1=iota[:],
                            op=ALU.is_equal,
                        )
                        ohs.append(oh)
                        ohT_ps = ps.tile([128, 128], fp32, tag="ohT")
                        nc.tensor.transpose(ohT_ps[:], oh[:], ident[:])
                        ohT = sb.tile([128, 128], fp32, tag="ohTs")
                        nc.scalar.activation(ohT[:], ohT_ps[:], ACT.Copy)
                        nc.tensor.matmul(
                            adeps[:, j * 8 : (j + 1) * 8],
                            lhsT=ohT[:], rhs=adblk[:, b, 0:8],
                            start=True, stop=True,
                        )
                    w = sb.tile([128, cn, 8], fp32, tag="w")
                    nc.vector.tensor_tensor(
                        out=w[:],
                        in0=hg[:, :, HC : HC + 8],
                        in1=adeps[:].rearrange("p (c e) -> p c e", e=8),
                        op=ALU.add,
                    )
                    wn = sb.tile([128, cn, 8], fp32, tag="wn")
                    nc.vector.tensor_scalar_mul(wn[:], w[:], NEG)
                    nc.vector.tensor_tensor(out=w[:], in0=w[:], in1=wn[:], op=ALU.max)
                    nc.scalar.activation(w[:], w[:], ACT.Exp)
                    msg = sb.tile([128, cn, HC + 8], fp32, tag="msg")
                    nc.vector.tensor_tensor(
                        out=msg[:, :, 0:HC].rearrange("p c (h y) -> p c h y", y=CH),
                        in0=hg[:, :, 0:HC].rearrange("p c (h y) -> p c h y", y=CH),
                        in1=w[:].unsqueeze(3).to_broadcast([128, cn, 8, CH]),
                        op=ALU.mult,
                    )
                    nc.vector.tensor_copy(out=msg[:, :, HC : HC + 8], in_=w[:])
                    for j in range(cn):
                        nc.tensor.matmul(
                            agg[:], lhsT=ohs[j][:], rhs=msg[:, j, :],
                            start=(t0 + j == 0), stop=(t0 + j == tmax - 1),
                        )
                    t0 += cn
                # finalize block: y1 = agg/Z + b1; h2 = ELU(y1)
                zc = sb.tile([128, 8], fp32, tag="zc")
                nc.vector.tensor_scalar_max(zc[:], agg[:, HC : HC + 8], 1e-30)
                zr = sb.tile([128, 8], fp32, tag="zr")
                nc.vector.reciprocal(zr[:], zc[:])
                y1 = h2res[:, b, :]
                nc.vector.tensor_tensor(
                    out=y1.rearrange("p (h y) -> p h y", y=CH),
                    in0=agg[:, 0:HC].rearrange("p (h y) -> p h y", y=CH),
                    in1=zr[:].unsqueeze(2).to_broadcast([128, 8, CH]),
                    op=ALU.mult,
                )
                nc.vector.tensor_tensor(out=y1, in0=y1, in1=b1r[:], op=ALU.add)
                el = sb.tile([128, HC], fp32, tag="el")
                nc.vector.tensor_scalar_min(el[:], y1, 0.0)
                nc.scalar.activation(el[:], el[:], ACT.Exp)
                nc.vector.tensor_scalar_max(y1, y1, 0.0)
                nc.vector.tensor_tensor(out=y1, in0=y1, in1=el[:], op=ALU.add)
                nc.vector.tensor_scalar_add(y1, y1, -1.0)

            ps.release()
            ps = tc.alloc_tile_pool(name="ps_g", bufs=2, space="PSUM")
            # ================= g table + AllGather =================
            for b in range(BPC):
                gps = ps.tile([128, CLS + 2], fp32, tag="gps")
                for h in range(2):
                    hTp = ps.tile([128, 128], fp32, tag="hTp")
                    nc.tensor.transpose(
                        hTp[:], h2res[:, b, h * 128 : (h + 1) * 128], ident[:]
                    )
                    hT = sb.tile([128, 128], fp32, tag="hTs")
                    nc.scalar.activation(hT[:], hTp[:], ACT.Copy)
                    nc.tensor.matmul(
                        gps[:], lhsT=hT[:], rhs=rhs2[:, h, :],
                        start=(h == 0), stop=(h == 1),
                    )
                gp = sb.tile([128, GP_W], fp32, tag="gp")
                nc.scalar.activation(gp[:, 0 : CLS + 2], gps[:], ACT.Copy)
                nc.vector.memset(gp[:, CLS + 2 : GP_W], 0.0)
                nc.sync.dma_start(gpk_in_t[b * 128 : (b + 1) * 128, :], gp[:])

            nc.gpsimd.collective_compute(
                "AllGather",
                mybir.AluOpType.bypass,
                replica_groups=[list(range(CORES))],
                ins=[gpk_in_t[:]],
                outs=[gpk_t[:]],
            )

            ad2blk = res.tile([128, BPC, GP_W], fp32)
            for p in range(NPC // GS):
                nc.gpsimd.dma_gather(
                    ad2blk[:, p * (GS // 128) : (p + 1) * (GS // 128), :], gpk_t[:],
                    gblk[:, p * (GS // 16) : (p + 1) * (GS // 16)], GS, GS, GP_W,
                    queue_num=0,
                )

            ps.release()
            ps = tc.alloc_tile_pool(name="ps_l2", bufs=2, space="PSUM")
            # ================= layer 2 edge phase =================
            for b in range(BPC):
                agg2 = ps.tile([128, CLS + 1], fp32, tag="agg2")
                dstl = sb.tile([128, tmax], fp32, tag="dstl")
                nc.sync.dma_start(dstl[:], gdstl_t[:, b * tmax : (b + 1) * tmax])
                t0 = 0
                for cn in chunks:
                    col0 = (b * tmax + t0) * 8
                    g2 = sb.tile([128, cn, GP_W], fp32, tag="g2")
                    nc.gpsimd.dma_gather(
                        g2[:], gpk_t[:], gsrc[:, col0 : col0 + cn * 8],
                        cn * 128, cnk_reg, GP_W, queue_num=0,
                    )
                    ohs = []
                    adeps2 = ps.tile([128, cn], fp32, tag="adeps2")
                    for j in range(cn):
                        oh = ohp.tile([128, 128], fp32, tag="oh")
                        nc.vector.tensor_tensor(
                            out=oh[:],
                            in0=dstl[:, t0 + j : t0 + j + 1].to_broadcast([128, 128]),
                            in1=iota[:],
                            op=ALU.is_equal,
                        )
                        ohs.append(oh)
                        ohT_ps = ps.tile([128, 128], fp32, tag="ohT")
                        nc.tensor.transpose(ohT_ps[:], oh[:], ident[:])
                        ohT = sb.tile([128, 128], fp32, tag="ohTs")
                        nc.scalar.activation(ohT[:], ohT_ps[:], ACT.Copy)
                        nc.tensor.matmul(
                            adeps2[:, j : j + 1],
                            lhsT=ohT[:], rhs=ad2blk[:, b, CLS + 1 : CLS + 2],
                            start=True, stop=True,
                        )
                    w2 = sb.tile([128, cn, 1], fp32, tag="w2")
                    nc.vector.tensor_tensor(
                        out=w2[:],
                        in0=g2[:, :, CLS : CLS + 1],
                        in1=adeps2[:].unsqueeze(2),
                        op=ALU.add,
                    )
                    w2n = sb.tile([128, cn, 1], fp32, tag="w2n")
                    nc.vector.tensor_scalar_mul(w2n[:], w2[:], NEG)
                    nc.vector.tensor_tensor(out=w2[:], in0=w2[:], in1=w2n[:], op=ALU.max)
                    nc.scalar.activation(w2[:], w2[:], ACT.Exp)
                    msg2 = sb.tile([128, cn, CLS + 1], fp32, tag="msg2")
                    nc.vector.tensor_tensor(
                        out=msg2[:, :, 0:CLS],
                        in0=g2[:, :, 0:CLS],
                        in1=w2[:].to_broadcast([128, cn, CLS]),
                        op=ALU.mult,
                    )
                    nc.vector.tensor_copy(out=msg2[:, :, CLS : CLS + 1], in_=w2[:])
                    for j in range(cn):
                        nc.tensor.matmul(
                            agg2[:], lhsT=ohs[j][:], rhs=msg2[:, j, :],
                            start=(t0 + j == 0), stop=(t0 + j == tmax - 1),
                        )
                    t0 += cn
                # finalize: y2 = agg2/Z + b2 -> log_softmax -> out
                z2c = sb.tile([128, 1], fp32, tag="z2c")
                nc.vector.tensor_scalar_max(z2c[:], agg2[:, CLS : CLS + 1], 1e-30)
                z2 = sb.tile([128, 1], fp32, tag="z2")
                nc.vector.reciprocal(z2[:], z2c[:])
                y2 = sb.tile([128, CLS], fp32, tag="y2")
                nc.vector.tensor_scalar(
                    out=y2[:], in0=agg2[:, 0:CLS], scalar1=z2[:, 0:1], scalar2=None,
                    op0=ALU.mult,
                )
                nc.vector.tensor_tensor(out=y2[:], in0=y2[:], in1=b2r[:], op=ALU.add)
                mx = sb.tile([128, 1], fp32, tag="mx")
                nc.vector.reduce_max(mx[:], y2[:], axis=mybir.AxisListType.X)
                nc.vector.tensor_scalar(
                    out=y2[:], in0=y2[:], scalar1=mx[:, 0:1], scalar2=None,
                    op0=ALU.subtract,
                )
                es = sb.tile([128, CLS], fp32, tag="es")
                ssum = sb.tile([128, 1], fp32, tag="ssum")
                nc.scalar.activation(es[:], y2[:], ACT.Exp, accum_out=ssum[:])
                lse = sb.tile([128, 1], fp32, tag="lse")
                nc.scalar.activation(lse[:], ssum[:], ACT.Ln)
                ob = sb.tile([128, CLS], fp32, tag="ob")
                nc.vector.tensor_scalar(
                    out=ob[:], in0=y2[:], scalar1=lse[:, 0:1], scalar2=None,
                    op0=ALU.subtract,
                )
                nc.sync.dma_start(out_t[b * 128 : (b + 1) * 128, :], ob[:])
            ps.release()

    nc.finalize()
    return nc


def _host_inputs(inputs, tmax, chunks, per_core):
    x = np.asarray(inputs["x"], dtype=np.float32)
    W1 = np.asarray(inputs["W1"], dtype=np.float32)
    a1s = np.asarray(inputs["a1_src"], dtype=np.float32)
    a1d = np.asarray(inputs["a1_dst"], dtype=np.float32)
    b1 = np.asarray(inputs["b1"], dtype=np.float32)
    W2 = np.asarray(inputs["W2"], dtype=np.float32)
    a2s = np.asarray(inputs["a2_src"], dtype=np.float32)
    a2d = np.asarray(inputs["a2_dst"], dtype=np.float32)
    b2 = np.asarray(inputs["b2"], dtype=np.float32)

    xpad = np.zeros((NPAD, F), dtype=np.float32)
    xpad[:N] = x
    ablk = np.zeros((HC, 16), dtype=np.float32)
    for h in range(HEADS):
        ablk[h * CH : (h + 1) * CH, h] = a1s[h]
        ablk[h * CH : (h + 1) * CH, 8 + h] = a1d[h]
    a2sd = np.stack([a2s[0], a2d[0]], axis=1).astype(np.float32)  # [40,2]
    common = {
        "x": xpad,
        "W1": np.ascontiguousarray(W1),
        "Ablk": ablk,
        "b1rep": np.tile(b1[None, :], (128, 1)).astype(np.float32),
        "W2": np.ascontiguousarray(W2),
        "W2T": np.ascontiguousarray(W2.T),
        "a2sd": a2sd,
        "b2rep": np.tile(b2[None, :], (128, 1)).astype(np.float32),
        "ident": np.eye(128, dtype=np.float32),
        "iotaF": np.tile(np.arange(128, dtype=np.float32)[None, :], (128, 1)),
    }
    maps = []
    for k in range(CORES):
        m = dict(common)
        m["gsrc"] = per_core[k]["gsrc"]
        m["gdstl"] = per_core[k]["gdstl"]
        m["gblk"] = per_core[k]["gblk"]
        maps.append(m)
    return maps


def kernel(**inputs):
    from concourse.bass_utils import run_bass_kernel_spmd

    edge_index = np.asarray(inputs["edge_index"])
    tmax, chunks, per_core = _prep_edges(edge_index)

    key = (tmax, tuple(chunks))
    if key not in _cache:
        _cache[key] = _build_nc(tmax, chunks)
    nc = _cache[key]

    in_maps = _host_inputs(inputs, tmax, chunks, per_core)
    res = run_bass_kernel_spmd(nc, in_maps, core_ids=list(range(CORES)))
    outs = [res.results[k]["out"] for k in range(CORES)]
    full = np.concatenate(outs, axis=0)[:N]
    return full.astype(np.float32)



# revision 4
# speedup vs baseline: 7.6486x; 7.6486x over previous
"""GAT (2-layer, PyG GATConv semantics) on 8 Trainium2 NeuronCores.

Strategy (dst-sharded edge parallelism, transfer/program-size optimized):
  - Append self-loops, sort edges by dst. Core k owns dst nodes
    [k*2560, (k+1)*2560) (N padded 20000 -> 20480), as 20 blocks of 128.
  - x is node-sharded (bf16): each core computes h = x@W1 (+ fused
    attention-logit columns) for its own 2560 nodes only, then one
    AllGather builds the full packed row table on every core's HBM.
  - Edge processing gathers h[src_e] rows with dma_gather, builds per-tile
    one-hot matrices from dst_local indices, and uses PE matmuls to
    (a) broadcast alpha_dst[dst] to edges and (b) scatter-add
    softmax-weighted messages + denominators into PSUM.
  - Softmax without max-subtraction (logits are O(1); identical math).
  - Layer loops are For_i hardware loops (20 iterations) with per-block
    staging DMAs so the program stays small (fast per-call jit/compile).
  - All weights/constants ship as one [16, 908] f32 shard per core,
    AllGathered on device; gather indices ship compact [16, .] int16 and
    are partition-replicated on device; dst-locals ship uint8.
"""

import math

import numpy as np

# ---- problem constants (hardcoded per contract) ----
N = 20000
F = 128
HEADS = 8
CH = 32
HC = HEADS * CH  # 256
CLS = 40
NEG = 0.2
CORES = 8
BLK = 128
BPC = 20  # blocks per core
NPC = BLK * BPC  # 2560 nodes per core
NPAD = NPC * CORES  # 20480
HP_W = 320  # packed h row: [h(256) | a_src(8) | a_dst(8) | pad] -> 1280B
GP_W = 64  # packed g row: [g(40) | as2(1) | ad2(1) | pad] -> 256B
CN = 7  # gather chunk size (tiles of 128 edges)

# wconst column layout
WC_RE = 0  # rhs_ext [W1 | U]           272
WC_R2 = WC_RE + HC + 16  # rhs2 halves  2*42
WC_B1 = WC_R2 + 2 * (CLS + 2)  # b1rep   256
WC_B2 = WC_B1 + HC  # b2rep              40
WC_ID = WC_B2 + CLS  # ident            128
WC_IO = WC_ID + 128  # iota             128
WC_W = WC_IO + 128  # 908

_cache = {}


def _wrap_idx16(idx):
    """dma_gather index layout, compact: [16, len//16] int16, idx i at
    [i%16, i//16] (device replicates to the 8 gpsimd partition groups)."""
    assert len(idx) % 16 == 0
    return np.ascontiguousarray(idx.astype(np.int16).reshape(-1, 16).T)


def _prep_edges(edge_index):
    src = np.asarray(edge_index[0], dtype=np.int64)
    dst = np.asarray(edge_index[1], dtype=np.int64)
    loops = np.arange(N, dtype=np.int64)
    src = np.concatenate([src, loops])
    dst = np.concatenate([dst, loops])
    order = np.argsort(dst, kind="stable")
    ssrc = src[order]
    sdst = dst[order]

    nblocks = NPAD // BLK  # 160
    counts = np.bincount(sdst // BLK, minlength=nblocks)
    starts = np.concatenate([[0], np.cumsum(counts)])
    # uniform CN-tile chunks (single num_idxs constant -> one gpsimd register)
    tmax = CN * int(math.ceil(counts.max() / 128 / CN))
    chunks = [CN] * (tmax // CN)

    per_core = []
    for k in range(CORES):
        gsrc_cols = []
        dstl_cols = np.empty((BPC * tmax, 128), dtype=np.uint8)
        for b in range(BPC):
            g = k * BPC + b
            e0, e1 = starts[g], starts[g + 1]
            npadded = tmax * 128
            s = np.zeros(npadded, dtype=np.int64)
            dl = np.full(npadded, 128, dtype=np.uint8)  # 128 = dead sentinel
            s[: e1 - e0] = ssrc[e0:e1]
            dl[: e1 - e0] = (sdst[e0:e1] - g * BLK).astype(np.uint8)
            dstl_cols[b * tmax : (b + 1) * tmax] = dl.reshape(tmax, 128)
            t0 = 0
            for cn in chunks:
                gsrc_cols.append(_wrap_idx16(s[t0 * 128 : (t0 + cn) * 128]))
                t0 += cn
        gsrc = np.concatenate(gsrc_cols, axis=1)  # [16, BPC*tmax*8]
        gdstl = np.ascontiguousarray(dstl_cols.T)  # [128, BPC*tmax] u8
        per_core.append({"gsrc": gsrc, "gdstl": gdstl})
    return tmax, chunks, per_core


def _build_nc(tmax, chunks):
    import concourse.bacc as bacc
    import concourse.bass as bass
    import concourse.mybir as mybir
    import concourse.tile as tile

    ds = bass.ds
    fp32 = mybir.dt.float32
    bf16 = mybir.dt.bfloat16
    i16 = mybir.dt.int16
    u8 = mybir.dt.uint8
    ALU = mybir.AluOpType
    ACT = mybir.ActivationFunctionType

    nc = bacc.Bacc("TRN2", target_bir_lowering=False, num_swdge_queues=4)

    L = BPC * tmax  # edge-tile columns per core

    # ---- I/O ----
    xbf_t = nc.dram_tensor("xbf", [NPC, F], bf16, kind="ExternalInput")
    wc_in_t = nc.dram_tensor("wcin", [16, WC_W], fp32, kind="ExternalInput")
    gsrc_t = nc.dram_tensor("gsrc", [16, L * 8], i16, kind="ExternalInput")
    gdstl_t = nc.dram_tensor("gdstl", [128, L], u8, kind="ExternalInput")
    out_t = nc.dram_tensor("out", [NPC, CLS], fp32, kind="ExternalOutput")

    wc_st_t = nc.dram_tensor("wcst", [16, WC_W], fp32)
    wc_sh_t = nc.dram_tensor("wcsh", [128, WC_W], fp32, addr_space="Shared")
    hpk_in_t = nc.dram_tensor("hpkin", [NPC, HP_W], fp32)
    hpk_t = nc.dram_tensor("hpk", [NPAD, HP_W], fp32, addr_space="Shared")
    gpk_in_t = nc.dram_tensor("gpkin", [NPC, GP_W], fp32)
    gpk_t = nc.dram_tensor("gpk", [NPAD, GP_W], fp32, addr_space="Shared")

    with tile.TileContext(nc) as tc:
        with (
            tc.tile_pool(name="const", bufs=1) as cp,
            tc.tile_pool(name="sb", bufs=2) as sb,
            tc.tile_pool(name="oh", bufs=2 * CN) as ohp,
            tc.tile_pool(name="res", bufs=1) as res,
        ):
            # ---- constants: AllGather the weight shard, load tables ----
            nc.sync.dma_start(wc_st_t[:], wc_in_t[:])
            nc.gpsimd.collective_compute(
                "AllGather",
                mybir.AluOpType.bypass,
                replica_groups=[list(range(CORES))],
                ins=[wc_st_t[:]],
                outs=[wc_sh_t[:]],
            )
            wct = cp.tile([128, WC_W], fp32)
            nc.sync.dma_start(wct[:], wc_sh_t[:])
            rhs_ext = wct[:, WC_RE : WC_RE + HC + 16]
            rhs2 = [
                wct[:, WC_R2 : WC_R2 + CLS + 2],
                wct[:, WC_R2 + CLS + 2 : WC_R2 + 2 * (CLS + 2)],
            ]
            b1r = wct[:, WC_B1 : WC_B1 + HC]
            b2r = wct[:, WC_B2 : WC_B2 + CLS]
            ident = wct[:, WC_ID : WC_ID + 128]
            iota = wct[:, WC_IO : WC_IO + 128]

            gsrc = cp.tile([128, L * 8], i16)
            nc.sync.dma_start(gsrc[0:16, :], gsrc_t[:])
            nc.sync.dma_start(gsrc[16:32, :], gsrc[0:16, :])
            nc.sync.dma_start(gsrc[32:64, :], gsrc[0:32, :])
            nc.sync.dma_start(gsrc[64:128, :], gsrc[0:64, :])

            gd8 = cp.tile([128, L], u8)
            nc.sync.dma_start(gd8[:], gdstl_t[:])
            gdf = cp.tile([128, L], fp32)
            nc.vector.tensor_copy(out=gdf[:], in_=gd8[:])

            ad_res = res.tile([128, BPC * 8], fp32)
            ad2_res = res.tile([128, BPC], fp32)
            h2res = res.tile([128, BPC * HC], fp32)
            cnk_reg = nc.gpsimd.to_reg(CN * 128)

            # ---- prologue: own-shard h | a_s | a_d -> hpk_in, ad_res ----
            ps = tc.alloc_tile_pool(name="ps_pro", bufs=2, space="PSUM")
            for t in range(BPC):
                xb = sb.tile([128, F], bf16, tag="xb")
                nc.sync.dma_start(xb[:], xbf_t[t * 128 : (t + 1) * 128, :])
                xf = sb.tile([128, F], fp32, tag="xf")
                nc.vector.tensor_copy(out=xf[:], in_=xb[:])
                xT_ps = ps.tile([128, 128], fp32, tag="xT")
                nc.tensor.transpose(xT_ps[:], xf[:], ident)
                xT = sb.tile([128, 128], fp32, tag="xTs")
                nc.vector.tensor_copy(out=xT[:], in_=xT_ps[:])
                hps = ps.tile([128, HC + 16], fp32, tag="hps")
                nc.tensor.matmul(hps[:], lhsT=xT[:], rhs=rhs_ext, start=True, stop=True)
                hp = sb.tile([128, HP_W], fp32, tag="hp")
                nc.vector.tensor_copy(out=hp[:, 0 : HC + 16], in_=hps[:])
                nc.vector.memset(hp[:, HC + 16 : HP_W], 0.0)
                nc.vector.tensor_copy(
                    out=ad_res[:, t * 8 : (t + 1) * 8], in_=hps[:, HC + 8 : HC + 16]
                )
                nc.sync.dma_start(hpk_in_t[t * 128 : (t + 1) * 128, :], hp[:])

            nc.gpsimd.collective_compute(
                "AllGather",
                mybir.AluOpType.bypass,
                replica_groups=[list(range(CORES))],
                ins=[hpk_in_t[:]],
                outs=[hpk_t[:]],
            )

            ps.release()
            ps = tc.alloc_tile_pool(name="ps_l1", bufs=2, space="PSUM")

            # ================= layer 1 edge phase =================
            with tc.For_i(0, BPC, 1) as i:
                bsrc = sb.tile([128, tmax * 8], i16, tag="bsrc")
                nc.sync.dma_start(bsrc[:], gsrc[:, ds(i * (tmax * 8), tmax * 8)])
                dstlc = sb.tile([128, tmax], fp32, tag="dstlc")
                nc.sync.dma_start(dstlc[:], gdf[:, ds(i * tmax, tmax)])
                adcur = sb.tile([128, 8], fp32, tag="adcur")
                nc.sync.dma_start(adcur[:], ad_res[:, ds(i * 8, 8)])

                agg = ps.tile([128, HC + 8], fp32, tag="agg")
                for c, cn in enumerate(chunks):
                    t0 = c * CN
                    hg = sb.tile([128, cn, HP_W], fp32, tag="hg")
                    nc.gpsimd.dma_gather(
                        hg[:], hpk_t[:], bsrc[:, t0 * 8 : (t0 + cn) * 8],
                        cn * 128, cnk_reg, HP_W, queue_num=0,
                    )
                    ohs = []
                    adeps = ps.tile([128, cn * 8], fp32, tag="adeps")
                    for j in range(cn):
                        oh = ohp.tile([128, 128], fp32, tag="oh")
                        nc.vector.tensor_tensor(
                            out=oh[:],
                            in0=dstlc[:, t0 + j : t0 + j + 1].to_broadcast([128, 128]),
                            in1=iota,
                            op=ALU.is_equal,
                        )
                        ohs.append(oh)
                        ohT_ps = ps.tile([128, 128], fp32, tag="ohT")
                        nc.tensor.transpose(ohT_ps[:], oh[:], ident)
                        ohT = sb.tile([128, 128], fp32, tag="ohTs")
                        nc.vector.tensor_copy(out=ohT[:], in_=ohT_ps[:])
                        nc.tensor.matmul(
                            adeps[:, j * 8 : (j + 1) * 8],
                            lhsT=ohT[:], rhs=adcur[:],
                            start=True, stop=True,
                        )
                    w = sb.tile([128, cn, 8], fp32, tag="w")
                    nc.vector.tensor_tensor(
                        out=w[:],
                        in0=hg[:, :, HC : HC + 8],
                        in1=adeps[:].rearrange("p (c e) -> p c e", e=8),
                        op=ALU.add,
                    )
                    wn = sb.tile([128, cn, 8], fp32, tag="wn")
                    nc.vector.tensor_scalar_mul(wn[:], w[:], NEG)
                    nc.vector.tensor_tensor(out=w[:], in0=w[:], in1=wn[:], op=ALU.max)
                    nc.scalar.activation(w[:], w[:], ACT.Exp)
                    msg = sb.tile([128, cn, HC + 8], fp32, tag="msg")
                    nc.vector.tensor_tensor(
                        out=msg[:, :, 0:HC].rearrange("p c (h y) -> p c h y", y=CH),
                        in0=hg[:, :, 0:HC].rearrange("p c (h y) -> p c h y", y=CH),
                        in1=w[:].unsqueeze(3).to_broadcast([128, cn, 8, CH]),
                        op=ALU.mult,
                    )
                    nc.vector.tensor_copy(out=msg[:, :, HC : HC + 8], in_=w[:])
                    for j in range(cn):
                        nc.tensor.matmul(
                            agg[:], lhsT=ohs[j][:], rhs=msg[:, j, :],
                            start=(t0 + j == 0), stop=(t0 + j == tmax - 1),
                        )
                # finalize block: y1 = agg/Z + b1; h2 = ELU(y1)
                zc = sb.tile([128, 8], fp32, tag="zc")
                nc.vector.tensor_scalar_max(zc[:], agg[:, HC : HC + 8], 1e-30)
                zr = sb.tile([128, 8], fp32, tag="zr")
                nc.vector.reciprocal(zr[:], zc[:])
                y1 = sb.tile([128, HC], fp32, tag="y1")
                nc.vector.tensor_tensor(
                    out=y1[:].rearrange("p (h y) -> p h y", y=CH),
                    in0=agg[:, 0:HC].rearrange("p (h y) -> p h y", y=CH),
                    in1=zr[:].unsqueeze(2).to_broadcast([128, 8, CH]),
                    op=ALU.mult,
                )
                nc.vector.tensor_tensor(out=y1[:], in0=y1[:], in1=b1r, op=ALU.add)
                el = sb.tile([128, HC], fp32, tag="el")
                nc.vector.tensor_scalar_min(el[:], y1[:], 0.0)
                nc.scalar.activation(el[:], el[:], ACT.Exp)
                nc.vector.tensor_scalar_max(y1[:], y1[:], 0.0)
                nc.vector.tensor_tensor(out=y1[:], in0=y1[:], in1=el[:], op=ALU.add)
                nc.vector.tensor_scalar_add(y1[:], y1[:], -1.0)
                nc.sync.dma_start(h2res[:, ds(i * HC, HC)], y1[:])

            ps.release()
            ps = tc.alloc_tile_pool(name="ps_g", bufs=2, space="PSUM")
            # ================= g table + AllGather =================
            for b in range(BPC):
                gps = ps.tile([128, CLS + 2], fp32, tag="gps")
                for h in range(2):
                    hTp = ps.tile([128, 128], fp32, tag="hTp")
                    nc.tensor.transpose(
                        hTp[:], h2res[:, b * HC + h * 128 : b * HC + (h + 1) * 128],
                        ident,
                    )
                    hT = sb.tile([128, 128], fp32, tag="hTs")
                    nc.vector.tensor_copy(out=hT[:], in_=hTp[:])
                    nc.tensor.matmul(
                        gps[:], lhsT=hT[:], rhs=rhs2[h],
                        start=(h == 0), stop=(h == 1),
                    )
                gp = sb.tile([128, GP_W], fp32, tag="gp")
                nc.vector.tensor_copy(out=gp[:, 0 : CLS + 2], in_=gps[:])
                nc.vector.memset(gp[:, CLS + 2 : GP_W], 0.0)
                nc.vector.tensor_copy(
                    out=ad2_res[:, b : b + 1], in_=gps[:, CLS + 1 : CLS + 2]
                )
                nc.sync.dma_start(gpk_in_t[b * 128 : (b + 1) * 128, :], gp[:])

            nc.gpsimd.collective_compute(
                "AllGather",
                mybir.AluOpType.bypass,
                replica_groups=[list(range(CORES))],
                ins=[gpk_in_t[:]],
                outs=[gpk_t[:]],
            )

            ps.release()
            ps = tc.alloc_tile_pool(name="ps_l2", bufs=2, space="PSUM")
            # ================= layer 2 edge phase =================
            with tc.For_i(0, BPC, 1) as i:
                bsrc = sb.tile([128, tmax * 8], i16, tag="bsrc")
                nc.sync.dma_start(bsrc[:], gsrc[:, ds(i * (tmax * 8), tmax * 8)])
                dstlc = sb.tile([128, tmax], fp32, tag="dstlc")
                nc.sync.dma_start(dstlc[:], gdf[:, ds(i * tmax, tmax)])
                ad2cur = sb.tile([128, 1], fp32, tag="ad2cur")
                nc.sync.dma_start(ad2cur[:], ad2_res[:, ds(i, 1)])

                agg2 = ps.tile([128, CLS + 1], fp32, tag="agg2")
                for c, cn in enumerate(chunks):
                    t0 = c * CN
                    g2 = sb.tile([128, cn, GP_W], fp32, tag="g2")
                    nc.gpsimd.dma_gather(
                        g2[:], gpk_t[:], bsrc[:, t0 * 8 : (t0 + cn) * 8],
                        cn * 128, cnk_reg, GP_W, queue_num=0,
                    )
                    ohs = []
                    adeps2 = ps.tile([128, cn], fp32, tag="adeps2")
                    for j in range(cn):
                        oh = ohp.tile([128, 128], fp32, tag="oh")
                        nc.vector.tensor_tensor(
                            out=oh[:],
                            in0=dstlc[:, t0 + j : t0 + j + 1].to_broadcast([128, 128]),
                            in1=iota,
                            op=ALU.is_equal,
                        )
                        ohs.append(oh)
                        ohT_ps = ps.tile([128, 128], fp32, tag="ohT")
                        nc.tensor.transpose(ohT_ps[:], oh[:], ident)
                        ohT = sb.tile([128, 128], fp32, tag="ohTs")
                        nc.vector.tensor_copy(out=ohT[:], in_=ohT_ps[:])
                        nc.tensor.matmul(
                            adeps2[:, j : j + 1],
                            lhsT=ohT[:], rhs=ad2cur[:],
                            start=True, stop=True,
                        )
                    w2 = sb.tile([128, cn, 1], fp32, tag="w2")
                    nc.vector.tensor_tensor(
                        out=w2[:],
                        in0=g2[:, :, CLS : CLS + 1],
                        in1=adeps2[:].unsqueeze(2),
                        op=ALU.add,
                    )
                    w2n = sb.tile([128, cn, 1], fp32, tag="w2n")
                    nc.vector.tensor_scalar_mul(w2n[:], w2[:], NEG)
                    nc.vector.tensor_tensor(out=w2[:], in0=w2[:], in1=w2n[:], op=ALU.max)
                    nc.scalar.activation(w2[:], w2[:], ACT.Exp)
                    msg2 = sb.tile([128, cn, CLS + 1], fp32, tag="msg2")
                    nc.vector.tensor_tensor(
                        out=msg2[:, :, 0:CLS],
                        in0=g2[:, :, 0:CLS],
                        in1=w2[:].to_broadcast([128, cn, CLS]),
                        op=ALU.mult,
                    )
                    nc.vector.tensor_copy(out=msg2[:, :, CLS : CLS + 1], in_=w2[:])
                    for j in range(cn):
                        nc.tensor.matmul(
                            agg2[:], lhsT=ohs[j][:], rhs=msg2[:, j, :],
                            start=(t0 + j == 0), stop=(t0 + j == tmax - 1),
                        )
                # finalize: y2 = agg2/Z + b2 -> log_softmax -> out
                z2c = sb.tile([128, 1], fp32, tag="z2c")
                nc.vector.tensor_scalar_max(z2c[:], agg2[:, CLS : CLS + 1], 1e-30)
                z2 = sb.tile([128, 1], fp32, tag="z2")
                nc.vector.reciprocal(z2[:], z2c[:])
                y2 = sb.tile([128, CLS], fp32, tag="y2")
                nc.vector.tensor_scalar(
                    out=y2[:], in0=agg2[:, 0:CLS], scalar1=z2[:, 0:1], scalar2=None,
                    op0=ALU.mult,
                )
                nc.vector.tensor_tensor(out=y2[:], in0=y2[:], in1=b2r, op=ALU.add)
                mx = sb.tile([128, 1], fp32, tag="mx")
                nc.vector.reduce_max(mx[:], y2[:], axis=mybir.AxisListType.X)
                nc.vector.tensor_scalar(
                    out=y2[:], in0=y2[:], scalar1=mx[:, 0:1], scalar2=None,
                    op0=ALU.subtract,
                )
                es = sb.tile([128, CLS], fp32, tag="es")
                ssum = sb.tile([128, 1], fp32, tag="ssum")
                nc.scalar.activation(es[:], y2[:], ACT.Exp, accum_out=ssum[:])
                lse = sb.tile([128, 1], fp32, tag="lse")
                nc.scalar.activation(lse[:], ssum[:], ACT.Ln)
                ob = sb.tile([128, CLS], fp32, tag="ob")
                nc.vector.tensor_scalar(
                    out=ob[:], in0=y2[:], scalar1=lse[:, 0:1], scalar2=None,
                    op0=ALU.subtract,
                )
                nc.sync.dma_start(out_t[ds(i * 128, 128), :], ob[:])
            ps.release()

    nc.finalize()
    return nc


def _host_inputs(inputs, tmax, chunks, per_core):
    import ml_dtypes

    x = np.asarray(inputs["x"], dtype=np.float32)
    W1 = np.asarray(inputs["W1"], dtype=np.float32)
    a1s = np.asarray(inputs["a1_src"], dtype=np.float32)
    a1d = np.asarray(inputs["a1_dst"], dtype=np.float32)
    b1 = np.asarray(inputs["b1"], dtype=np.float32)
    W2 = np.asarray(inputs["W2"], dtype=np.float32)
    a2s = np.asarray(inputs["a2_src"], dtype=np.float32)
    a2d = np.asarray(inputs["a2_dst"], dtype=np.float32)
    b2 = np.asarray(inputs["b2"], dtype=np.float32)

    xpad = np.zeros((NPAD, F), dtype=np.float32)
    xpad[:N] = x
    xbf = xpad.astype(ml_dtypes.bfloat16)

    ablk = np.zeros((HC, 16), dtype=np.float32)
    for h in range(HEADS):
        ablk[h * CH : (h + 1) * CH, h] = a1s[h]
        ablk[h * CH : (h + 1) * CH, 8 + h] = a1d[h]

    wc = np.zeros((128, WC_W), dtype=np.float32)
    wc[:, WC_RE : WC_RE + HC] = W1
    wc[:, WC_RE + HC : WC_RE + HC + 16] = W1 @ ablk
    for h in range(2):
        c0 = WC_R2 + h * (CLS + 2)
        Wh = W2[h * 128 : (h + 1) * 128, :]
        wc[:, c0 : c0 + CLS] = Wh
        wc[:, c0 + CLS] = Wh @ a2s[0]
        wc[:, c0 + CLS + 1] = Wh @ a2d[0]
    wc[:, WC_B1 : WC_B1 + HC] = b1[None, :]
    wc[:, WC_B2 : WC_B2 + CLS] = b2[None, :]
    wc[:, WC_ID : WC_ID + 128] = np.eye(128, dtype=np.float32)
    wc[:, WC_IO : WC_IO + 128] = np.arange(128, dtype=np.float32)[None, :]

    maps = []
    for k in range(CORES):
        maps.append(
            {
                "xbf": np.ascontiguousarray(xbf[k * NPC : (k + 1) * NPC]),
                "wcin": np.ascontiguousarray(wc[k * 16 : (k + 1) * 16]),
                "gsrc": per_core[k]["gsrc"],
                "gdstl": per_core[k]["gdstl"],
            }
        )
    return maps


def kernel(**inputs):
    from concourse.bass_utils import run_bass_kernel_spmd

    edge_index = np.asarray(inputs["edge_index"])
    tmax, chunks, per_core = _prep_edges(edge_index)

    key = (tmax, tuple(chunks))
    if key not in _cache:
        _cache[key] = _build_nc(tmax, chunks)
    nc = _cache[key]

    in_maps = _host_inputs(inputs, tmax, chunks, per_core)
    res = run_bass_kernel_spmd(nc, in_maps, core_ids=list(range(CORES)))
    outs = [res.results[k]["out"] for k in range(CORES)]
    full = np.concatenate(outs, axis=0)[:N]
    return full.astype(np.float32)


# revision 11
# speedup vs baseline: 8.5898x; 1.1230x over previous
"""GAT (2-layer, PyG GATConv semantics) on 8 Trainium2 NeuronCores.

Strategy (dst-sharded edge parallelism, transfer/program-size optimized):
  - Append self-loops, sort edges by dst. Core k owns dst nodes
    [k*2560, (k+1)*2560) (N padded 20000 -> 20480), as 20 blocks of 128.
  - x is node-sharded (bf16): each core computes h = x@W1 (+ fused
    attention-logit columns) for its own 2560 nodes only, then one
    AllGather builds the full packed row table on every core's HBM.
  - Edge processing gathers h[src_e] rows with dma_gather, builds per-tile
    one-hot matrices from dst_local indices, and uses PE matmuls to
    (a) broadcast alpha_dst[dst] to edges and (b) scatter-add
    softmax-weighted messages + denominators into PSUM.
  - Softmax without max-subtraction (logits are O(1); identical math).
  - Layer loops are For_i hardware loops (20 iterations) with per-block
    staging DMAs so the program stays small (fast per-call jit/compile).
  - All weights/constants ship as one [16, 908] f32 shard per core,
    AllGathered on device; gather indices ship compact [16, .] int16 and
    are partition-replicated on device; dst-locals ship uint8.
"""

import math

import numpy as np

# ---- problem constants (hardcoded per contract) ----
N = 20000
F = 128
HEADS = 8
CH = 32
HC = HEADS * CH  # 256
CLS = 40
NEG = 0.2
CORES = 8
BLK = 128
BPC = 20  # blocks per core
NPC = BLK * BPC  # 2560 nodes per core
NPAD = NPC * CORES  # 20480
HP_W = 320  # packed h row: [h(256) | a_src(8) | a_dst(8) | pad] -> 1280B
GP_W = 64  # packed g row: [g(40) | as2(1) | ad2(1) | pad] -> 256B
CN = 7  # gather chunk size (tiles of 128 edges)

# wconst column layout
WC_RE = 0  # rhs_ext [W1 | U]           272
WC_R2 = WC_RE + HC + 16  # rhs2 halves  2*42
WC_B1 = WC_R2 + 2 * (CLS + 2)  # b1rep   256
WC_B2 = WC_B1 + HC  # b2rep              40
WC_ID = WC_B2 + CLS  # ident            128
WC_IO = WC_ID + 128  # iota             128
WC_W = WC_IO + 128  # 908

_cache = {}


def _wrap_idx16(idx):
    """dma_gather index layout, compact: [16, len//16] int16, idx i at
    [i%16, i//16] (device replicates to the 8 gpsimd partition groups)."""
    assert len(idx) % 16 == 0
    return np.ascontiguousarray(idx.astype(np.int16).reshape(-1, 16).T)


def _prep_edges(edge_index):
    src = np.asarray(edge_index[0], dtype=np.int64)
    dst = np.asarray(edge_index[1], dtype=np.int64)
    loops = np.arange(N, dtype=np.int64)
    src = np.concatenate([src, loops])
    dst = np.concatenate([dst, loops])
    order = np.argsort(dst, kind="stable")
    ssrc = src[order]
    sdst = dst[order]

    nblocks = NPAD // BLK  # 160
    counts = np.bincount(sdst // BLK, minlength=nblocks)
    starts = np.concatenate([[0], np.cumsum(counts)])
    # uniform CN-tile chunks (single num_idxs constant -> one gpsimd register)
    tmax = CN * int(math.ceil(counts.max() / 128 / CN))
    chunks = [CN] * (tmax // CN)

    per_core = []
    for k in range(CORES):
        gsrc_cols = []
        dstl_cols = np.empty((BPC * tmax, 128), dtype=np.uint8)
        for b in range(BPC):
            g = k * BPC + b
            e0, e1 = starts[g], starts[g + 1]
            npadded = tmax * 128
            s = np.zeros(npadded, dtype=np.int64)
            dl = np.full(npadded, 128, dtype=np.uint8)  # 128 = dead sentinel
            s[: e1 - e0] = ssrc[e0:e1]
            dl[: e1 - e0] = (sdst[e0:e1] - g * BLK).astype(np.uint8)
            dstl_cols[b * tmax : (b + 1) * tmax] = dl.reshape(tmax, 128)
            t0 = 0
            for cn in chunks:
                gsrc_cols.append(_wrap_idx16(s[t0 * 128 : (t0 + cn) * 128]))
                t0 += cn
        gsrc = np.concatenate(gsrc_cols, axis=1)  # [16, BPC*tmax*8]
        gdstl = np.ascontiguousarray(dstl_cols.T)  # [128, BPC*tmax] u8
        per_core.append({"gsrc": gsrc, "gdstl": gdstl})
    return tmax, chunks, per_core


def _build_nc(tmax, chunks):
    import concourse.bacc as bacc
    import concourse.bass as bass
    import concourse.mybir as mybir
    import concourse.tile as tile

    ds = bass.ds
    fp32 = mybir.dt.float32
    bf16 = mybir.dt.bfloat16
    i16 = mybir.dt.int16
    u8 = mybir.dt.uint8
    ALU = mybir.AluOpType
    ACT = mybir.ActivationFunctionType

    nc = bacc.Bacc("TRN2", target_bir_lowering=False, num_swdge_queues=4)

    L = BPC * tmax  # edge-tile columns per core

    # ---- I/O ----
    xbf_t = nc.dram_tensor("xbf", [NPC, F], bf16, kind="ExternalInput")
    wc_in_t = nc.dram_tensor("wcin", [16, WC_W], fp32, kind="ExternalInput")
    gsrc_t = nc.dram_tensor("gsrc", [16, L * 8], i16, kind="ExternalInput")
    gdstl_t = nc.dram_tensor("gdstl", [128, L], u8, kind="ExternalInput")
    out_t = nc.dram_tensor("out", [NPC, CLS], bf16, kind="ExternalOutput")

    wc_st_t = nc.dram_tensor("wcst", [16, WC_W], fp32)
    wc_sh_t = nc.dram_tensor("wcsh", [128, WC_W], fp32, addr_space="Shared")
    hpk_in_t = nc.dram_tensor("hpkin", [NPC, HP_W], fp32)
    hpk_t = nc.dram_tensor("hpk", [NPAD, HP_W], fp32, addr_space="Shared")
    gpk_in_t = nc.dram_tensor("gpkin", [NPC, GP_W], fp32)
    gpk_t = nc.dram_tensor("gpk", [NPAD, GP_W], fp32, addr_space="Shared")

    with tile.TileContext(nc) as tc:
        with (
            tc.tile_pool(name="const", bufs=1) as cp,
            tc.tile_pool(name="sb", bufs=2) as sb,
            tc.tile_pool(name="oh", bufs=2 * CN) as ohp,
        ):
            # ---- constants: AllGather the weight shard, load tables ----
            nc.sync.dma_start(wc_st_t[:], wc_in_t[:])
            nc.gpsimd.collective_compute(
                "AllGather",
                mybir.AluOpType.bypass,
                replica_groups=[list(range(CORES))],
                ins=[wc_st_t[:]],
                outs=[wc_sh_t[:]],
            )
            wct = cp.tile([128, WC_W], fp32)
            nc.sync.dma_start(wct[:], wc_sh_t[:])
            rhs_ext = wct[:, WC_RE : WC_RE + HC + 16]
            rhs2 = [
                wct[:, WC_R2 : WC_R2 + CLS + 2],
                wct[:, WC_R2 + CLS + 2 : WC_R2 + 2 * (CLS + 2)],
            ]
            b1r = wct[:, WC_B1 : WC_B1 + HC]
            b2r = wct[:, WC_B2 : WC_B2 + CLS]
            ident = wct[:, WC_ID : WC_ID + 128]
            iota = wct[:, WC_IO : WC_IO + 128]

            gsrc = cp.tile([128, L * 8], i16)
            nc.sync.dma_start(gsrc[0:16, :], gsrc_t[:])
            nc.sync.dma_start(gsrc[16:32, :], gsrc[0:16, :])
            nc.sync.dma_start(gsrc[32:64, :], gsrc[0:32, :])
            nc.sync.dma_start(gsrc[64:128, :], gsrc[0:64, :])

            gd8 = cp.tile([128, L], u8)
            nc.sync.dma_start(gd8[:], gdstl_t[:])
            gdf = cp.tile([128, L], fp32)
            nc.vector.tensor_copy(out=gdf[:], in_=gd8[:])

            cnk_reg = nc.gpsimd.to_reg(CN * 128)

            # ---- prologue: own-shard h | a_s | a_d -> hpk_in ----
            ps = tc.alloc_tile_pool(name="ps_pro", bufs=2, space="PSUM")
            with tc.For_i(0, BPC, 1) as i:
                xb = sb.tile([128, F], bf16, tag="xb")
                nc.sync.dma_start(xb[:], xbf_t[ds(i * 128, 128), :])
                xf = sb.tile([128, F], fp32, tag="xf")
                nc.vector.tensor_copy(out=xf[:], in_=xb[:])
                xT_ps = ps.tile([128, 128], fp32, tag="xT")
                nc.tensor.transpose(xT_ps[:], xf[:], ident)
                xT = sb.tile([128, 128], fp32, tag="xTs")
                nc.vector.tensor_copy(out=xT[:], in_=xT_ps[:])
                hps = ps.tile([128, HC + 16], fp32, tag="hps")
                nc.tensor.matmul(hps[:], lhsT=xT[:], rhs=rhs_ext, start=True, stop=True)
                hp = sb.tile([128, HP_W], fp32, tag="hp")
                nc.vector.tensor_copy(out=hp[:, 0 : HC + 16], in_=hps[:])
                nc.vector.memset(hp[:, HC + 16 : HP_W], 0.0)
                nc.sync.dma_start(hpk_in_t[ds(i * 128, 128), :], hp[:])

            nc.gpsimd.collective_compute(
                "AllGather",
                mybir.AluOpType.bypass,
                replica_groups=[list(range(CORES))],
                ins=[hpk_in_t[:]],
                outs=[hpk_t[:]],
            )

            ps.release()
            ps = tc.alloc_tile_pool(name="ps_l1", bufs=2, space="PSUM")
            psg = tc.alloc_tile_pool(name="ps_l1g", bufs=1, space="PSUM")

            # ================= layer 1 edge phase (+ g table) =================
            with tc.For_i(0, BPC, 1) as i:
                bsrc = sb.tile([128, tmax * 8], i16, tag="bsrc")
                nc.sync.dma_start(bsrc[:], gsrc[:, ds(i * (tmax * 8), tmax * 8)])
                dstlc = sb.tile([128, tmax], fp32, tag="dstlc")
                nc.sync.dma_start(dstlc[:], gdf[:, ds(i * tmax, tmax)])
                adcur = sb.tile([128, 8], fp32, tag="adcur")
                nc.sync.dma_start(adcur[:], hpk_in_t[ds(i * 128, 128), HC + 8 : HC + 16])

                agg = ps.tile([128, HC + 8], fp32, tag="agg")
                for c, cn in enumerate(chunks):
                    t0 = c * CN
                    hg = sb.tile([128, cn, HP_W], fp32, tag="hg")
                    nc.gpsimd.dma_gather(
                        hg[:], hpk_t[:], bsrc[:, t0 * 8 : (t0 + cn) * 8],
                        cn * 128, cnk_reg, HP_W, queue_num=0,
                    )
                    ohs = []
                    adeps = ps.tile([128, cn * 8], fp32, tag="adeps")
                    for j in range(cn):
                        oh = ohp.tile([128, 128], fp32, tag="oh")
                        nc.vector.tensor_tensor(
                            out=oh[:],
                            in0=dstlc[:, t0 + j : t0 + j + 1].to_broadcast([128, 128]),
                            in1=iota,
                            op=ALU.is_equal,
                        )
                        ohs.append(oh)
                        ohT_ps = ps.tile([128, 128], fp32, tag="ohT")
                        nc.tensor.transpose(ohT_ps[:], oh[:], ident)
                        ohT = sb.tile([128, 128], fp32, tag="ohTs")
                        nc.vector.tensor_copy(out=ohT[:], in_=ohT_ps[:])
                        nc.tensor.matmul(
                            adeps[:, j * 8 : (j + 1) * 8],
                            lhsT=ohT[:], rhs=adcur[:],
                            start=True, stop=True,
                        )
                    w = sb.tile([128, cn, 8], fp32, tag="w")
                    nc.vector.tensor_tensor(
                        out=w[:],
                        in0=hg[:, :, HC : HC + 8],
                        in1=adeps[:].rearrange("p (c e) -> p c e", e=8),
                        op=ALU.add,
                    )
                    wn = sb.tile([128, cn, 8], fp32, tag="wn")
                    nc.vector.tensor_scalar_mul(wn[:], w[:], NEG)
                    nc.vector.tensor_tensor(out=w[:], in0=w[:], in1=wn[:], op=ALU.max)
                    nc.scalar.activation(w[:], w[:], ACT.Exp)
                    msg = sb.tile([128, cn, HC + 8], fp32, tag="msg")
                    nc.vector.tensor_tensor(
                        out=msg[:, :, 0:HC].rearrange("p c (h y) -> p c h y", y=CH),
                        in0=hg[:, :, 0:HC].rearrange("p c (h y) -> p c h y", y=CH),
                        in1=w[:].unsqueeze(3).to_broadcast([128, cn, 8, CH]),
                        op=ALU.mult,
                    )
                    nc.vector.tensor_copy(out=msg[:, :, HC : HC + 8], in_=w[:])
                    for j in range(cn):
                        nc.tensor.matmul(
                            agg[:], lhsT=ohs[j][:], rhs=msg[:, j, :],
                            start=(t0 + j == 0), stop=(t0 + j == tmax - 1),
                        )
                # finalize block: y1 = agg/Z + b1; h2 = ELU(y1)
                zc = sb.tile([128, 8], fp32, tag="zc")
                nc.vector.tensor_scalar_max(zc[:], agg[:, HC : HC + 8], 1e-30)
                zr = sb.tile([128, 8], fp32, tag="zr")
                nc.vector.reciprocal(zr[:], zc[:])
                y1 = sb.tile([128, HC], fp32, tag="y1")
                nc.vector.tensor_tensor(
                    out=y1[:].rearrange("p (h y) -> p h y", y=CH),
                    in0=agg[:, 0:HC].rearrange("p (h y) -> p h y", y=CH),
                    in1=zr[:].unsqueeze(2).to_broadcast([128, 8, CH]),
                    op=ALU.mult,
                )
                nc.vector.tensor_tensor(out=y1[:], in0=y1[:], in1=b1r, op=ALU.add)
                el = sb.tile([128, HC], fp32, tag="el")
                nc.vector.tensor_scalar_min(el[:], y1[:], 0.0)
                nc.scalar.activation(el[:], el[:], ACT.Exp)
                nc.vector.tensor_scalar_max(y1[:], y1[:], 0.0)
                nc.vector.tensor_tensor(out=y1[:], in0=y1[:], in1=el[:], op=ALU.add)
                nc.vector.tensor_scalar_add(y1[:], y1[:], -1.0)
                # g table for this block
                gps = psg.tile([128, CLS + 2], fp32, tag="gps")
                for h in range(2):
                    hTp = ps.tile([128, 128], fp32, tag="ohT")
                    nc.tensor.transpose(
                        hTp[:], y1[:, h * 128 : (h + 1) * 128], ident
                    )
                    hT = sb.tile([128, 128], fp32, tag="ohTs")
                    nc.vector.tensor_copy(out=hT[:], in_=hTp[:])
                    nc.tensor.matmul(
                        gps[:], lhsT=hT[:], rhs=rhs2[h],
                        start=(h == 0), stop=(h == 1),
                    )
                gp = sb.tile([128, GP_W], fp32, tag="gp")
                nc.vector.tensor_copy(out=gp[:, 0 : CLS + 2], in_=gps[:])
                nc.vector.memset(gp[:, CLS + 2 : GP_W], 0.0)
                nc.sync.dma_start(gpk_in_t[ds(i * 128, 128), :], gp[:])

            psg.release()

            nc.gpsimd.collective_compute(
                "AllGather",
                mybir.AluOpType.bypass,
                replica_groups=[list(range(CORES))],
                ins=[gpk_in_t[:]],
                outs=[gpk_t[:]],
            )

            ps.release()
            ps = tc.alloc_tile_pool(name="ps_l2", bufs=2, space="PSUM")
            # ================= layer 2 edge phase =================
            with tc.For_i(0, BPC, 1) as i:
                bsrc = sb.tile([128, tmax * 8], i16, tag="bsrc")
                nc.sync.dma_start(bsrc[:], gsrc[:, ds(i * (tmax * 8), tmax * 8)])
                dstlc = sb.tile([128, tmax], fp32, tag="dstlc")
                nc.sync.dma_start(dstlc[:], gdf[:, ds(i * tmax, tmax)])
                ad2cur = sb.tile([128, 1], fp32, tag="ad2cur")
                nc.sync.dma_start(
                    ad2cur[:], gpk_in_t[ds(i * 128, 128), CLS + 1 : CLS + 2]
                )

                agg2 = ps.tile([128, CLS + 1], fp32, tag="agg2")
                for c, cn in enumerate(chunks):
                    t0 = c * CN
                    g2 = sb.tile([128, cn, GP_W], fp32, tag="g2")
                    nc.gpsimd.dma_gather(
                        g2[:], gpk_t[:], bsrc[:, t0 * 8 : (t0 + cn) * 8],
                        cn * 128, cnk_reg, GP_W, queue_num=0,
                    )
                    ohs = []
                    adeps2 = ps.tile([128, cn], fp32, tag="adeps2")
                    for j in range(cn):
                        oh = ohp.tile([128, 128], fp32, tag="oh")
                        nc.vector.tensor_tensor(
                            out=oh[:],
                            in0=dstlc[:, t0 + j : t0 + j + 1].to_broadcast([128, 128]),
                            in1=iota,
                            op=ALU.is_equal,
                        )
                        ohs.append(oh)
                        ohT_ps = ps.tile([128, 128], fp32, tag="ohT")
                        nc.tensor.transpose(ohT_ps[:], oh[:], ident)
                        ohT = sb.tile([128, 128], fp32, tag="ohTs")
                        nc.vector.tensor_copy(out=ohT[:], in_=ohT_ps[:])
                        nc.tensor.matmul(
                            adeps2[:, j : j + 1],
                            lhsT=ohT[:], rhs=ad2cur[:],
                            start=True, stop=True,
                        )
                    w2 = sb.tile([128, cn, 1], fp32, tag="w2")
                    nc.vector.tensor_tensor(
                        out=w2[:],
                        in0=g2[:, :, CLS : CLS + 1],
                        in1=adeps2[:].unsqueeze(2),
                        op=ALU.add,
                    )
                    w2n = sb.tile([128, cn, 1], fp32, tag="w2n")
                    nc.vector.tensor_scalar_mul(w2n[:], w2[:], NEG)
                    nc.vector.tensor_tensor(out=w2[:], in0=w2[:], in1=w2n[:], op=ALU.max)
                    nc.scalar.activation(w2[:], w2[:], ACT.Exp)
                    msg2 = sb.tile([128, cn, CLS + 1], fp32, tag="msg2")
                    nc.vector.tensor_tensor(
                        out=msg2[:, :, 0:CLS],
                        in0=g2[:, :, 0:CLS],
                        in1=w2[:].to_broadcast([128, cn, CLS]),
                        op=ALU.mult,
                    )
                    nc.vector.tensor_copy(out=msg2[:, :, CLS : CLS + 1], in_=w2[:])
                    for j in range(cn):
                        nc.tensor.matmul(
                            agg2[:], lhsT=ohs[j][:], rhs=msg2[:, j, :],
                            start=(t0 + j == 0), stop=(t0 + j == tmax - 1),
                        )
                # finalize: y2 = agg2/Z + b2 -> log_softmax -> out
                z2c = sb.tile([128, 1], fp32, tag="z2c")
                nc.vector.tensor_scalar_max(z2c[:], agg2[:, CLS : CLS + 1], 1e-30)
                z2 = sb.tile([128, 1], fp32, tag="z2")
                nc.vector.reciprocal(z2[:], z2c[:])
                y2 = sb.tile([128, CLS], fp32, tag="y2")
                nc.vector.tensor_scalar(
                    out=y2[:], in0=agg2[:, 0:CLS], scalar1=z2[:, 0:1], scalar2=None,
                    op0=ALU.mult,
                )
                nc.vector.tensor_tensor(out=y2[:], in0=y2[:], in1=b2r, op=ALU.add)
                mx = sb.tile([128, 1], fp32, tag="mx")
                nc.vector.reduce_max(mx[:], y2[:], axis=mybir.AxisListType.X)
                nc.vector.tensor_scalar(
                    out=y2[:], in0=y2[:], scalar1=mx[:, 0:1], scalar2=None,
                    op0=ALU.subtract,
                )
                es = sb.tile([128, CLS], fp32, tag="es")
                ssum = sb.tile([128, 1], fp32, tag="ssum")
                nc.scalar.activation(es[:], y2[:], ACT.Exp, accum_out=ssum[:])
                lse = sb.tile([128, 1], fp32, tag="lse")
                nc.scalar.activation(lse[:], ssum[:], ACT.Ln)
                ob = sb.tile([128, CLS], bf16, tag="ob")
                nc.vector.tensor_scalar(
                    out=ob[:], in0=y2[:], scalar1=lse[:, 0:1], scalar2=None,
                    op0=ALU.subtract,
                )
                nc.sync.dma_start(out_t[ds(i * 128, 128), :], ob[:])
            ps.release()

    nc.finalize()
    return nc


def _host_inputs(inputs, tmax, chunks, per_core):
    import ml_dtypes

    x = np.asarray(inputs["x"], dtype=np.float32)
    W1 = np.asarray(inputs["W1"], dtype=np.float32)
    a1s = np.asarray(inputs["a1_src"], dtype=np.float32)
    a1d = np.asarray(inputs["a1_dst"], dtype=np.float32)
    b1 = np.asarray(inputs["b1"], dtype=np.float32)
    W2 = np.asarray(inputs["W2"], dtype=np.float32)
    a2s = np.asarray(inputs["a2_src"], dtype=np.float32)
    a2d = np.asarray(inputs["a2_dst"], dtype=np.float32)
    b2 = np.asarray(inputs["b2"], dtype=np.float32)

    xpad = np.zeros((NPAD, F), dtype=np.float32)
    xpad[:N] = x
    xbf = xpad.astype(ml_dtypes.bfloat16)

    ablk = np.zeros((HC, 16), dtype=np.float32)
    for h in range(HEADS):
        ablk[h * CH : (h + 1) * CH, h] = a1s[h]
        ablk[h * CH : (h + 1) * CH, 8 + h] = a1d[h]

    wc = np.zeros((128, WC_W), dtype=np.float32)
    wc[:, WC_RE : WC_RE + HC] = W1
    wc[:, WC_RE + HC : WC_RE + HC + 16] = W1 @ ablk
    for h in range(2):
        c0 = WC_R2 + h * (CLS + 2)
        Wh = W2[h * 128 : (h + 1) * 128, :]
        wc[:, c0 : c0 + CLS] = Wh
        wc[:, c0 + CLS] = Wh @ a2s[0]
        wc[:, c0 + CLS + 1] = Wh @ a2d[0]
    wc[:, WC_B1 : WC_B1 + HC] = b1[None, :]
    wc[:, WC_B2 : WC_B2 + CLS] = b2[None, :]
    wc[:, WC_ID : WC_ID + 128] = np.eye(128, dtype=np.float32)
    wc[:, WC_IO : WC_IO + 128] = np.arange(128, dtype=np.float32)[None, :]

    maps = []
    for k in range(CORES):
        maps.append(
            {
                "xbf": np.ascontiguousarray(xbf[k * NPC : (k + 1) * NPC]),
                "wcin": np.ascontiguousarray(wc[k * 16 : (k + 1) * 16]),
                "gsrc": per_core[k]["gsrc"],
                "gdstl": per_core[k]["gdstl"],
            }
        )
    return maps


def kernel(**inputs):
    from concourse.bass_utils import run_bass_kernel_spmd

    edge_index = np.asarray(inputs["edge_index"])
    tmax, chunks, per_core = _prep_edges(edge_index)

    key = (tmax, tuple(chunks))
    if key not in _cache:
        _cache[key] = _build_nc(tmax, chunks)
    nc = _cache[key]

    in_maps = _host_inputs(inputs, tmax, chunks, per_core)
    res = run_bass_kernel_spmd(nc, in_maps, core_ids=list(range(CORES)))
    outs = [res.results[k]["out"] for k in range(CORES)]
    full = np.concatenate(outs, axis=0)[:N]
    return full.astype(np.float32)
